# revision 21
# baseline (speedup 1.0000x reference)
"""GAT layer kernel for Trainium2, 8 NeuronCores, batch-sharded, pair-batched.

Math (per graph g of B=128, M=512 nodes, in=128, out D=64):
    Wh = h @ W.T;  s_src = Wh @ a[:D];  s_dst = Wh @ a[D:]
    e[i,j] = leakyrelu_0.2(s_src[i] + s_dst[j])
    out = elu(softmax(e, -1) @ Wh)

Key identity: exp is monotone, so exp(leakyrelu(e)) = max(exp(e), exp(e/5)),
and e = s_src[m] + s_dst[n] makes both branches rank-1 separable. Softmax is
scale-invariant per column m, so the exp(s_src[m]) factor cancels, leaving
    P'[n, m] = max(b1[n], q[m] * b2[n])
with b1 = exp(s_dst), b2 = exp(0.2*s_dst) (per-partition scalars) and
q[m] = exp(-0.8*s_src[m]) replicated across partitions by a broadcast matmul
(wsb: all columns equal -0.8*w_src).

v2: graphs are processed in PAIRS so the fixed per-instruction overheads
(ACT ~180ns, DVE ~60-95ns, sem ~125ns) amortize over 2x the data:
  - ONE exp over the pair's two qb PSUM banks ([128, 2, 512], 1114ns vs
    2x720), ONE bcol exp, ONE wha copy (FD=512 strided, 694ns vs 2x471),
    ONE pair finale exp/relu.
  - PSUM tiles are pair-sized with bank-aligned halves: ps_wh/ps_o are
    [128, 2, NC, 128] f32 so each matmul target sits inside one 2KB bank.
  - ht in / out DMAs are one transfer per pair (host pre-pairs the layout).
  - finale: pair recip ([128,2,NC] in one op), pair x-mult TT (FD=512),
    elu(x) = min(exp(x)-1, relu(x)) with exp/relu on ACT and the stt on DVE;
    output bf16 (halves out DMA; elu output is O(1) so bf16 rel err ~0.4%).
Engine balance per pair (~ns, measured): DVE = 8x409 P' + 160 recip +
753 x + 687 stt ~= 4.9us; ACT = 1114 qexp + 274 bcol + 694 wha + 694 exp +
711 relu ~= 3.5us; PE = 42 matmuls ~= 3.4us. The loop is software-pipelined:
pair pk's finale runs inside iteration pk+1; pair pk+1's front matmuls are
hoisted ahead of pair pk's attention on PE. Host pre-permutes node order
(pi(i) = 4*(i%128) + i//128) so the output tile [128p, NC, 64] is
node-ordered after a host reshape and DMAs with 1KB contiguous lines.
"""

import os
import sys
import types
from contextlib import ExitStack

import numpy as np
import ml_dtypes

# Defensive: concourse.bass_utils imports antenv.axon_hooks when tracing is
# requested (BASS_TRACE). Some images lack that module; register a stub so a
# traced run degrades to untraced instead of crashing.
try:
    import antenv.axon_hooks  # noqa: F401
except Exception:
    try:
        import antenv

        _hooks = types.ModuleType("antenv.axon_hooks")
        _hooks._hook = None
        _hooks.set_axon_ntff_profile_hook = lambda h: setattr(_hooks, "_hook", h)
        _hooks.get_axon_ntff_profile_hook = lambda: _hooks._hook
        sys.modules["antenv.axon_hooks"] = _hooks
        antenv.axon_hooks = _hooks
    except Exception:
        pass

import concourse.bass as bass
import concourse.tile as tile
from concourse import bacc, mybir
from concourse._compat import with_exitstack
from concourse.bass import ds, ts
from concourse.bass_utils import run_bass_kernel_spmd

B, M, IN_DIM, D = 128, 512, 128, 64
N_CORES = 8
G = B // N_CORES  # graphs per core
NP = G // 2  # graph pairs per core
NC = M // 128  # 128-node chunks per graph
ALPHA = 0.2
F32 = mybir.dt.float32
BF16 = mybir.dt.bfloat16

LAST_RESULTS = None  # BassKernelResults of the most recent run (for test.py)


@with_exitstack
def _gat_body(ctx: ExitStack, tc: tile.TileContext, out_ap, ht_ap, wq_ap, wsb_ap):
    nc = tc.nc
    mult = mybir.AluOpType.mult
    amax = mybir.AluOpType.max
    amin = mybir.AluOpType.min
    aadd = mybir.AluOpType.add
    AF = mybir.ActivationFunctionType

    const = ctx.enter_context(tc.tile_pool(name="const", bufs=1))
    ht_pool = ctx.enter_context(tc.tile_pool(name="ht", bufs=3))
    q_pool = ctx.enter_context(tc.tile_pool(name="q", bufs=2))
    sc_pool = ctx.enter_context(tc.tile_pool(name="sc", bufs=2))
    wha_pool = ctx.enter_context(tc.tile_pool(name="wha", bufs=2))
    p_pool = ctx.enter_context(tc.tile_pool(name="p", bufs=2))
    fin_pool = ctx.enter_context(tc.tile_pool(name="fin", bufs=2))
    out_pool = ctx.enter_context(tc.tile_pool(name="out", bufs=2))
    ps_qb = ctx.enter_context(tc.tile_pool(name="ps_qb", bufs=1, space="PSUM"))
    ps_wh = ctx.enter_context(tc.tile_pool(name="ps_wh", bufs=1, space="PSUM"))
    ps_o = ctx.enter_context(tc.tile_pool(name="ps_o", bufs=2, space="PSUM"))

    # wsb gates the very first matmul (qb), so it heads the sync queue --
    # the earliest-ready DMA path; wq (needed slightly later) goes on gpsimd
    wsb_s = const.tile([IN_DIM, 128], BF16)
    nc.scalar.dma_start(wsb_s[:], wsb_ap[:])
    wq_s = const.tile([IN_DIM, D + 2], BF16)
    nc.gpsimd.dma_start(wq_s[:], wq_ap[:])

    # Dummy activation at kernel start: triggers the one-time ACT table
    # load while the first ht DMA is in flight.
    warm = const.tile([1, 16], F32)
    nc.scalar.memzero(warm[:])
    nc.scalar.activation(warm[:], warm[:], AF.Exp)

    ht_tiles = {}

    def fetch_ht(pk, split=False):
        t = ht_pool.tile([IN_DIM, 2, M], BF16)
        if split:  # per-graph DMAs, j0 on gpsimd: two DGE inits overlap
            nc.gpsimd.dma_start(t[:, 0, :], ht_ap[pk, :, 0, :])
            nc.sync.dma_start(t[:, 1, :], ht_ap[pk, :, 1, :])
        else:
            nc.sync.dma_start(t[:], ht_ap[pk])
        ht_tiles[pk] = t

    mm_state = {}

    def front_mms(pk, split=False):
        """qb + phase-1 matmuls for pair pk, hoisted ahead of pair pk-1's
        attention on the PE queue. split: graph-major order (prologue)."""
        ht_t = ht_tiles.pop(pk)
        qb_ps = ps_qb.tile([128, 2, M], F32)
        wh_ps = ps_wh.tile([128, 2, NC, 128], F32)
        jorder = [(j, kind) for j in range(2) for kind in ("qb", "ph1")] \
            if split else \
            [(0, "qb"), (1, "qb"), (0, "ph1"), (1, "ph1")]
        for j, kind in jorder:
            if kind == "qb":
                # qb[n, m] = -0.8*s_src[m] (all-columns-equal wsb): heads the
                # longest chain (qb -> qexp -> P' -> attention)
                nc.tensor.matmul(
                    qb_ps[:, j, :], wsb_s[:], ht_t[:, j, :],
                    start=True, stop=True,
                )
            else:
                for c in range(NC):
                    nc.tensor.matmul(
                        wh_ps[:, j, c, 0 : D + 2],
                        ht_t[:, j, ts(c, 128)],
                        wq_s[:],
                        start=True,
                        stop=True,
                    )
        mm_state[pk] = (qb_ps, wh_ps)

    fin = {}  # finale state per pair

    def fin_dve(pk, j=None):
        """recip + x-mult; x = p_o * (1/Z) in bf16. j=None: whole pair."""
        o_ps = fin[pk, "o_ps"]
        if (pk, "r4") not in fin:
            fin[pk, "r4"] = fin_pool.tile([128, 2, NC], F32, tag="r4", name="r4")
            fin[pk, "x4"] = fin_pool.tile(
                [128, 2, NC, D], BF16, tag="x4", name="x4"
            )
        r4, x4 = fin[pk, "r4"], fin[pk, "x4"]
        if j is None:
            nc.vector.reciprocal_approx_fast(r4[:], o_ps[:, :, :, D])
            r4b = r4[:].unsqueeze(3).broadcast_to([128, 2, NC, D])
            nc.vector.tensor_tensor(x4[:], o_ps[:, :, :, 0:D], r4b, mult)
        else:
            nc.vector.reciprocal_approx_fast(r4[:, j, :], o_ps[:, j, :, D])
            r4b = r4[:, j, :].unsqueeze(2).broadcast_to([128, NC, D])
            nc.vector.tensor_tensor(x4[:, j], o_ps[:, j, :, 0:D], r4b, mult)

    def fin_act(pk, j=None, relu_on_dve=False):
        """elu pieces: E = exp(x) on ACT; R = relu(x) on ACT, or on DVE
        (ts max-imm runs in 4x mode) to shorten the kernel-tail chain."""
        x4 = fin[pk, "x4"]
        if (pk, "er") not in fin:
            fin[pk, "er"] = (
                fin_pool.tile([128, 2, NC, D], BF16, tag="e4", name="e4"),
                fin_pool.tile([128, 2, NC, D], BF16, tag="r4t", name="r4t"),
            )
        e4, r4t = fin[pk, "er"]
        if j is None:
            nc.scalar.activation(e4[:], x4[:], AF.Exp)
            nc.scalar.activation(r4t[:], x4[:], AF.Relu)
        else:
            nc.scalar.activation(e4[:, j], x4[:, j], AF.Exp)
            if relu_on_dve:
                nc.vector.tensor_scalar(
                    r4t[:, j], x4[:, j], 0.0, None, amax
                )
            else:
                nc.scalar.activation(r4t[:, j], x4[:, j], AF.Relu)

    def fin_tail(pk, j=None):
        """elu(x) = min(exp(x) - 1, relu(x)); DMA out."""
        e4, r4t = fin[pk, "er"]
        if (pk, "o4") not in fin:
            fin[pk, "o4"] = out_pool.tile([128, 2, NC, D], BF16, name="o4")
        o4 = fin[pk, "o4"]
        if j is None:
            nc.vector.scalar_tensor_tensor(o4[:], e4[:], -1.0, r4t[:], aadd, amin)
            nc.sync.dma_start(out_ap[pk], o4[:])
            for k in ("o_ps", "r4", "x4", "er", "o4"):
                fin.pop((pk, k))
        else:
            nc.vector.scalar_tensor_tensor(
                o4[:, j], e4[:, j], -1.0, r4t[:, j], aadd, amin
            )
            nc.gpsimd.dma_start(out_ap[pk, :, j], o4[:, j])
            if j == 1:
                for k in ("o_ps", "r4", "x4", "er", "o4"):
                    fin.pop((pk, k))

    fetch_ht(0, split=True)
    fetch_ht(1)
    front_mms(0, split=True)
    for pk in range(NP):
        if pk + 2 < NP:
            fetch_ht(pk + 2)
        qb_ps, wh_ps = mm_state.pop(pk)

        # qrep[n, (j, m)] = exp(-0.8*s_src[m]) -- heads ACT queue
        qrep = q_pool.tile([128, 2, M], BF16)
        bcol = sc_pool.tile([128, 2, NC, 2], F32)
        if pk == 0:  # per-graph on the ramp so P' j0 starts ~2.5us earlier
            for j in range(2):
                nc.scalar.activation(qrep[:, j, :], qb_ps[:, j, :], AF.Exp)
                nc.scalar.activation(
                    bcol[:, j], wh_ps[:, j, :, D : D + 2], AF.Exp
                )
        else:
            # bcol first, then per-graph qexp: P' j0's inputs complete
            # ~0.6us earlier each window (DVE was input-starved at the
            # window head); costs +260ns/pair on ACT which has slack
            nc.scalar.activation(bcol[:], wh_ps[:, :, :, D : D + 2], AF.Exp)
            nc.scalar.activation(qrep[:, 0, :], qb_ps[:, 0, :], AF.Exp)
            nc.scalar.activation(qrep[:, 1, :], qb_ps[:, 1, :], AF.Exp)

        if pk >= 1:
            # finale exp/relu BEFORE the wha copy on ACT: nothing needs wha
            # until attention at window-end, but the stt needs e4/r4t
            # mid-window -- this removes the stt's ~1us input wait
            fin_dve(pk - 1)
            fin_act(pk - 1)

        # wha [.., 65] = [Wh | 1] bf16 (ones column -> softmax denominator)
        wha = wha_pool.tile([128, 2, NC, D + 1], BF16)
        nc.scalar.activation(wha[:, :, :, 0:D], wh_ps[:, :, :, 0:D], AF.Copy)
        nc.gpsimd.memset(wha[:, :, :, D : D + 1], 1.0)

        # P'[n, m] = max(q[m]*b2[n], b1[n]): one tensor_scalar per chunk
        p1 = p_pool.tile([128, 2, NC, M], BF16)
        for j in range(2):
            for c in range(NC):
                nc.vector.tensor_scalar(
                    p1[:, j, c, :],
                    qrep[:, j, :],
                    bcol[:, j, c, 1:2],
                    bcol[:, j, c, 0:1],
                    mult,
                    amax,
                )

        # next pair's front matmuls go ahead of this attention on PE
        if pk + 1 < NP:
            front_mms(pk + 1)

        if pk >= 1:
            fin_tail(pk - 1)

        # attention: p_o[m, 65] accumulated over chunks; col 64 = Z_m.
        # Both graphs' attention back-to-back (a finale emitted in between
        # would make att j1 wait on j0's finale reads of the shared pair
        # tile); the last pair's finale is inlined after, per graph, with
        # relu on DVE (4x imm mode) and the final stt+DMA split in halves
        # so the last exposed DMA transfer is small and issues early.
        last = pk == NP - 1
        o_ps = ps_o.tile([128, 2, NC, 128], F32)
        fin[pk, "o_ps"] = o_ps
        for j in range(2):
            for mc in range(NC):
                for c in range(NC):
                    nc.tensor.matmul(
                        o_ps[:, j, mc, 0 : D + 1],
                        p1[:, j, c, ds(mc * 128, 128)],
                        wha[:, j, c, :],
                        start=(c == 0),
                        stop=(c == NC - 1),
                    )
        if last:
            for j in range(2):
                fin_dve(pk, j=j)
                fin_act(pk, j=j, relu_on_dve=True)
                if j == 0:
                    fin_tail(pk, j=0)
            e4, r4t = fin[pk, "er"]
            o4 = fin[pk, "o4"]
            h = NC // 2
            for lo, hi in ((0, h), (h, NC)):
                nc.vector.scalar_tensor_tensor(
                    o4[:, 1, lo:hi], e4[:, 1, lo:hi], -1.0,
                    r4t[:, 1, lo:hi], aadd, amin,
                )
                nc.gpsimd.dma_start(out_ap[pk, :, 1, lo:hi], o4[:, 1, lo:hi])


_CACHE = {}


def _build():
    if "nc" in _CACHE:
        return _CACHE["nc"]
    nc = bacc.Bacc(
        "TRN2", target_bir_lowering=False, debug=False, num_devices=N_CORES
    )
    ht_d = nc.dram_tensor("ht", [NP, IN_DIM, 2, M], BF16, kind="ExternalInput")
    wq_d = nc.dram_tensor("wq", [IN_DIM, D + 2], BF16, kind="ExternalInput")
    wsb_d = nc.dram_tensor("wsb", [IN_DIM, 128], BF16, kind="ExternalInput")
    # out[pk, p, j, mc, :] = graph 2*pk+j, node 4*p + mc
    out_d = nc.dram_tensor("out", [NP, 128, 2, NC, D], BF16, kind="ExternalOutput")
    with tile.TileContext(nc) as tc:
        _gat_body(tc, out_d.ap(), ht_d.ap(), wq_d.ap(), wsb_d.ap())
    nc.compile()
    _CACHE["nc"] = nc
    return nc


# Device column i holds node pi(i) = 4*(i % 128) + i // 128, so that the
# attention output tile [128p, NC, D] is node-ordered after a host reshape
# (node = 4p + mc) and the output DMA has 1KB-contiguous lines.
_PERM = (np.arange(M) % 128) * NC + (np.arange(M) // 128)


def host_prep(h, W, a):
    wt = W.T.astype(np.float32)  # [128, 64]
    w_src = wt @ a[:D]
    w_dst = wt @ a[D:]
    wq = np.concatenate(
        [wt, w_dst[:, None], 0.2 * w_dst[:, None]], axis=1
    ).astype(ml_dtypes.bfloat16)  # [128, 66]
    wsb = np.ascontiguousarray(
        np.repeat((-0.8 * w_src)[:, None], 128, axis=1)
    ).astype(ml_dtypes.bfloat16)  # [128, 128], every column -0.8*w_src
    return wq, wsb


def kernel(h, W, a):
    global LAST_RESULTS
    h = np.asarray(h, dtype=np.float32)
    W = np.asarray(W, dtype=np.float32)
    a = np.asarray(a, dtype=np.float32)

    wq, wsb = host_prep(h, W, a)

    nc = _build()
    in_maps = []
    for c in range(N_CORES):
        h_c = h[c * G : (c + 1) * G]  # [G, 512, 128]
        ht_c = h_c[:, _PERM, :].transpose(0, 2, 1)  # [G, 128, 512]
        ht_p = np.ascontiguousarray(
            ht_c.reshape(NP, 2, IN_DIM, M).transpose(0, 2, 1, 3)
        ).astype(ml_dtypes.bfloat16)  # [NP, 128, 2, 512]
        in_maps.append({"ht": ht_p, "wq": wq, "wsb": wsb})

    res = run_bass_kernel_spmd(nc, in_maps, list(range(N_CORES)))
    LAST_RESULTS = res
    outs = []
    for r in res.results:
        o = np.asarray(r["out"]).astype(np.float32)  # [NP, 128, 2, NC, D]
        outs.append(o.transpose(0, 2, 1, 3, 4).reshape(G, M, D))
    return np.concatenate(outs, axis=0).astype(np.float32)


# revision 22
# speedup vs baseline: 1.0320x; 1.0320x over previous
"""GAT layer kernel for Trainium2, 8 NeuronCores, batch-sharded, pair-batched.

Math (per graph g of B=128, M=512 nodes, in=128, out D=64):
    Wh = h @ W.T;  s_src = Wh @ a[:D];  s_dst = Wh @ a[D:]
    e[i,j] = leakyrelu_0.2(s_src[i] + s_dst[j])
    out = elu(softmax(e, -1) @ Wh)

Key identity: exp is monotone, so exp(leakyrelu(e)) = max(exp(e), exp(e/5)),
and e = s_src[m] + s_dst[n] makes both branches rank-1 separable. Softmax is
scale-invariant per column m, so the exp(s_src[m]) factor cancels, leaving
    P'[n, m] = max(b1[n], q[m] * b2[n])
with b1 = exp(s_dst), b2 = exp(0.2*s_dst) (per-partition scalars) and
q[m] = exp(-0.8*s_src[m]) replicated across partitions by a broadcast matmul
(wsb: all columns equal -0.8*w_src).

v2: graphs are processed in PAIRS so the fixed per-instruction overheads
(ACT ~180ns, DVE ~60-95ns, sem ~125ns) amortize over 2x the data:
  - ONE exp over the pair's two qb PSUM banks ([128, 2, 512], 1114ns vs
    2x720), ONE bcol exp, ONE wha copy (FD=512 strided, 694ns vs 2x471),
    ONE pair finale exp/relu.
  - PSUM tiles are pair-sized with bank-aligned halves: ps_wh/ps_o are
    [128, 2, NC, 128] f32 so each matmul target sits inside one 2KB bank.
  - ht in / out DMAs are one transfer per pair (host pre-pairs the layout).
  - finale: pair recip ([128,2,NC] in one op), pair x-mult TT (FD=512),
    elu(x) = min(exp(x)-1, relu(x)) with exp/relu on ACT and the stt on DVE;
    output bf16 (halves out DMA; elu output is O(1) so bf16 rel err ~0.4%).
Engine balance per pair (~ns, measured): DVE = 8x409 P' + 160 recip +
753 x + 687 stt ~= 4.9us; ACT = 1114 qexp + 274 bcol + 694 wha + 694 exp +
711 relu ~= 3.5us; PE = 42 matmuls ~= 3.4us. The loop is software-pipelined:
pair pk's finale runs inside iteration pk+1; pair pk+1's front matmuls are
hoisted ahead of pair pk's attention on PE. Host pre-permutes node order
(pi(i) = 4*(i%128) + i//128) so the output tile [128p, NC, 64] is
node-ordered after a host reshape and DMAs with 1KB contiguous lines.
"""

import os
import sys
import types
from contextlib import ExitStack

import numpy as np
import ml_dtypes

# Defensive: concourse.bass_utils imports antenv.axon_hooks when tracing is
# requested (BASS_TRACE). Some images lack that module; register a stub so a
# traced run degrades to untraced instead of crashing.
try:
    import antenv.axon_hooks  # noqa: F401
except Exception:
    try:
        import antenv

        _hooks = types.ModuleType("antenv.axon_hooks")
        _hooks._hook = None
        _hooks.set_axon_ntff_profile_hook = lambda h: setattr(_hooks, "_hook", h)
        _hooks.get_axon_ntff_profile_hook = lambda: _hooks._hook
        sys.modules["antenv.axon_hooks"] = _hooks
        antenv.axon_hooks = _hooks
    except Exception:
        pass

import concourse.bass as bass
import concourse.tile as tile
from concourse import bacc, mybir
from concourse._compat import with_exitstack
from concourse.bass import ds, ts
from concourse.bass_utils import run_bass_kernel_spmd

B, M, IN_DIM, D = 128, 512, 128, 64
N_CORES = 8
G = B // N_CORES  # graphs per core
NP = G // 2  # graph pairs per core
NC = M // 128  # 128-node chunks per graph
ALPHA = 0.2
F32 = mybir.dt.float32
BF16 = mybir.dt.bfloat16

LAST_RESULTS = None  # BassKernelResults of the most recent run (for test.py)


@with_exitstack
def _gat_body(ctx: ExitStack, tc: tile.TileContext, out_ap, ht_ap, wq_ap, wsb_ap):
    nc = tc.nc
    mult = mybir.AluOpType.mult
    amax = mybir.AluOpType.max
    amin = mybir.AluOpType.min
    aadd = mybir.AluOpType.add
    AF = mybir.ActivationFunctionType

    const = ctx.enter_context(tc.tile_pool(name="const", bufs=1))
    ht_pool = ctx.enter_context(tc.tile_pool(name="ht", bufs=3))
    q_pool = ctx.enter_context(tc.tile_pool(name="q", bufs=2))
    sc_pool = ctx.enter_context(tc.tile_pool(name="sc", bufs=2))
    wha_pool = ctx.enter_context(tc.tile_pool(name="wha", bufs=2))
    p_pool = ctx.enter_context(tc.tile_pool(name="p", bufs=2))
    fin_pool = ctx.enter_context(tc.tile_pool(name="fin", bufs=2))
    out_pool = ctx.enter_context(tc.tile_pool(name="out", bufs=2))
    ps_qb = ctx.enter_context(tc.tile_pool(name="ps_qb", bufs=1, space="PSUM"))
    ps_wh = ctx.enter_context(tc.tile_pool(name="ps_wh", bufs=1, space="PSUM"))
    ps_o = ctx.enter_context(tc.tile_pool(name="ps_o", bufs=2, space="PSUM"))

    # wsb gates the very first matmul (qb), so it heads the sync queue --
    # the earliest-ready DMA path; wq (needed slightly later) goes on gpsimd
    wsb_s = const.tile([IN_DIM, 128], BF16)
    nc.scalar.dma_start(wsb_s[:], wsb_ap[:])
    wq_s = const.tile([IN_DIM, D + 2], BF16)
    nc.gpsimd.dma_start(wq_s[:], wq_ap[:])

    # Dummy activation at kernel start: triggers the one-time ACT table
    # load while the first ht DMA is in flight.
    warm = const.tile([1, 16], F32)
    nc.scalar.memzero(warm[:])
    nc.scalar.activation(warm[:], warm[:], AF.Exp)

    ht_tiles = {}

    def fetch_ht(pk, split=False):
        t = ht_pool.tile([IN_DIM, 2, M], BF16)
        if split:  # per-graph DMAs so pair 0's first qb starts earlier
            nc.sync.dma_start(t[:, 0, :], ht_ap[pk, :, 0, :])
            nc.sync.dma_start(t[:, 1, :], ht_ap[pk, :, 1, :])
        else:
            nc.sync.dma_start(t[:], ht_ap[pk])
        ht_tiles[pk] = t

    mm_state = {}

    def front_mms(pk, split=False):
        """qb + phase-1 matmuls for pair pk, hoisted ahead of pair pk-1's
        attention on the PE queue. split: graph-major order (prologue)."""
        ht_t = ht_tiles.pop(pk)
        qb_ps = ps_qb.tile([128, 2, M], F32)
        wh_ps = ps_wh.tile([128, 2, NC, 128], F32)
        jorder = [(j, kind) for j in range(2) for kind in ("qb", "ph1")] \
            if split else \
            [(0, "qb"), (1, "qb"), (0, "ph1"), (1, "ph1")]
        for j, kind in jorder:
            if kind == "qb":
                # qb[n, m] = -0.8*s_src[m] (all-columns-equal wsb): heads the
                # longest chain (qb -> qexp -> P' -> attention)
                nc.tensor.matmul(
                    qb_ps[:, j, :], wsb_s[:], ht_t[:, j, :],
                    start=True, stop=True,
                )
            else:
                for c in range(NC):
                    nc.tensor.matmul(
                        wh_ps[:, j, c, 0 : D + 2],
                        ht_t[:, j, ts(c, 128)],
                        wq_s[:],
                        start=True,
                        stop=True,
                    )
        mm_state[pk] = (qb_ps, wh_ps)

    fin = {}  # finale state per pair

    def fin_dve(pk, j=None):
        """recip + x-mult; x = p_o * (1/Z) in bf16. j=None: whole pair."""
        o_ps = fin[pk, "o_ps"]
        if (pk, "r4") not in fin:
            fin[pk, "r4"] = fin_pool.tile([128, 2, NC], F32, tag="r4", name="r4")
            fin[pk, "x4"] = fin_pool.tile(
                [128, 2, NC, D], BF16, tag="x4", name="x4"
            )
        r4, x4 = fin[pk, "r4"], fin[pk, "x4"]
        if j is None:
            nc.vector.reciprocal_approx_fast(r4[:], o_ps[:, :, :, D])
            r4b = r4[:].unsqueeze(3).broadcast_to([128, 2, NC, D])
            nc.vector.tensor_tensor(x4[:], o_ps[:, :, :, 0:D], r4b, mult)
        else:
            nc.vector.reciprocal_approx_fast(r4[:, j, :], o_ps[:, j, :, D])
            r4b = r4[:, j, :].unsqueeze(2).broadcast_to([128, NC, D])
            nc.vector.tensor_tensor(x4[:, j], o_ps[:, j, :, 0:D], r4b, mult)

    def fin_act(pk, j=None, relu_on_dve=False):
        """elu pieces: E = exp(x) on ACT; R = relu(x) on ACT, or on DVE
        (ts max-imm runs in 4x mode) to shorten the kernel-tail chain."""
        x4 = fin[pk, "x4"]
        if (pk, "er") not in fin:
            fin[pk, "er"] = (
                fin_pool.tile([128, 2, NC, D], BF16, tag="e4", name="e4"),
                fin_pool.tile([128, 2, NC, D], BF16, tag="r4t", name="r4t"),
            )
        e4, r4t = fin[pk, "er"]
        if j is None:
            nc.scalar.activation(e4[:], x4[:], AF.Exp)
            nc.scalar.activation(r4t[:], x4[:], AF.Relu)
        else:
            nc.scalar.activation(e4[:, j], x4[:, j], AF.Exp)
            if relu_on_dve:
                nc.vector.tensor_scalar(
                    r4t[:, j], x4[:, j], 0.0, None, amax
                )
            else:
                nc.scalar.activation(r4t[:, j], x4[:, j], AF.Relu)

    def fin_tail(pk, j=None):
        """elu(x) = min(exp(x) - 1, relu(x)); DMA out."""
        e4, r4t = fin[pk, "er"]
        if (pk, "o4") not in fin:
            fin[pk, "o4"] = out_pool.tile([128, 2, NC, D], BF16, name="o4")
        o4 = fin[pk, "o4"]
        if j is None:
            nc.vector.scalar_tensor_tensor(o4[:], e4[:], -1.0, r4t[:], aadd, amin)
            nc.sync.dma_start(out_ap[pk], o4[:])
            for k in ("o_ps", "r4", "x4", "er", "o4"):
                fin.pop((pk, k))
        else:
            nc.vector.scalar_tensor_tensor(
                o4[:, j], e4[:, j], -1.0, r4t[:, j], aadd, amin
            )
            nc.sync.dma_start(out_ap[pk, :, j], o4[:, j])
            if j == 1:
                for k in ("o_ps", "r4", "x4", "er", "o4"):
                    fin.pop((pk, k))

    fetch_ht(0, split=True)
    fetch_ht(1)
    front_mms(0, split=True)
    for pk in range(NP):
        if pk + 2 < NP:
            fetch_ht(pk + 2)
        qb_ps, wh_ps = mm_state.pop(pk)

        # qrep[n, (j, m)] = exp(-0.8*s_src[m]) -- heads ACT queue
        qrep = q_pool.tile([128, 2, M], BF16)
        bcol = sc_pool.tile([128, 2, NC, 2], F32)
        if pk == 0:  # per-graph on the ramp so P' j0 starts ~2.5us earlier
            for j in range(2):
                nc.scalar.activation(qrep[:, j, :], qb_ps[:, j, :], AF.Exp)
                nc.scalar.activation(
                    bcol[:, j], wh_ps[:, j, :, D : D + 2], AF.Exp
                )
        else:
            # bcol first, then per-graph qexp: P' j0's inputs complete
            # ~0.6us earlier each window (DVE was input-starved at the
            # window head); costs +260ns/pair on ACT which has slack
            nc.scalar.activation(bcol[:], wh_ps[:, :, :, D : D + 2], AF.Exp)
            nc.scalar.activation(qrep[:, 0, :], qb_ps[:, 0, :], AF.Exp)
            nc.scalar.activation(qrep[:, 1, :], qb_ps[:, 1, :], AF.Exp)

        if pk >= 1:
            # finale exp/relu BEFORE the wha copy on ACT: nothing needs wha
            # until attention at window-end, but the stt needs e4/r4t
            # mid-window -- this removes the stt's ~1us input wait
            fin_dve(pk - 1)
            fin_act(pk - 1)

        # wha [.., 65] = [Wh | 1] bf16 (ones column -> softmax denominator)
        wha = wha_pool.tile([128, 2, NC, D + 1], BF16)
        nc.scalar.activation(wha[:, :, :, 0:D], wh_ps[:, :, :, 0:D], AF.Copy)
        nc.gpsimd.memset(wha[:, :, :, D : D + 1], 1.0)

        # P'[n, m] = max(q[m]*b2[n], b1[n]): one tensor_scalar per chunk
        p1 = p_pool.tile([128, 2, NC, M], BF16)
        for j in range(2):
            for c in range(NC):
                nc.vector.tensor_scalar(
                    p1[:, j, c, :],
                    qrep[:, j, :],
                    bcol[:, j, c, 1:2],
                    bcol[:, j, c, 0:1],
                    mult,
                    amax,
                )

        # next pair's front matmuls go ahead of this attention on PE
        if pk + 1 < NP:
            front_mms(pk + 1)

        if pk >= 1:
            fin_tail(pk - 1)

        # attention: p_o[m, 65] accumulated over chunks; col 64 = Z_m.
        # Both graphs' attention back-to-back (a finale emitted in between
        # would make att j1 wait on j0's finale reads of the shared pair
        # tile); the last pair's finale is inlined after, per graph, with
        # relu on DVE (4x imm mode) and the final stt+DMA split in halves
        # so the last exposed DMA transfer is small and issues early.
        last = pk == NP - 1
        o_ps = ps_o.tile([128, 2, NC, 128], F32)
        fin[pk, "o_ps"] = o_ps
        for j in range(2):
            for mc in range(NC):
                for c in range(NC):
                    nc.tensor.matmul(
                        o_ps[:, j, mc, 0 : D + 1],
                        p1[:, j, c, ds(mc * 128, 128)],
                        wha[:, j, c, :],
                        start=(c == 0),
                        stop=(c == NC - 1),
                    )
        if last:
            for j in range(2):
                fin_dve(pk, j=j)
                fin_act(pk, j=j, relu_on_dve=True)
                if j == 0:
                    fin_tail(pk, j=0)
            e4, r4t = fin[pk, "er"]
            o4 = fin[pk, "o4"]
            h = NC // 2
            for lo, hi in ((0, h), (h, NC)):
                nc.vector.scalar_tensor_tensor(
                    o4[:, 1, lo:hi], e4[:, 1, lo:hi], -1.0,
                    r4t[:, 1, lo:hi], aadd, amin,
                )
                nc.sync.dma_start(out_ap[pk, :, 1, lo:hi], o4[:, 1, lo:hi])


_CACHE = {}


def _build():
    if "nc" in _CACHE:
        return _CACHE["nc"]
    nc = bacc.Bacc(
        "TRN2", target_bir_lowering=False, debug=False, num_devices=N_CORES
    )
    ht_d = nc.dram_tensor("ht", [NP, IN_DIM, 2, M], BF16, kind="ExternalInput")
    wq_d = nc.dram_tensor("wq", [IN_DIM, D + 2], BF16, kind="ExternalInput")
    wsb_d = nc.dram_tensor("wsb", [IN_DIM, 128], BF16, kind="ExternalInput")
    # out[pk, p, j, mc, :] = graph 2*pk+j, node 4*p + mc
    out_d = nc.dram_tensor("out", [NP, 128, 2, NC, D], BF16, kind="ExternalOutput")
    with tile.TileContext(nc) as tc:
        _gat_body(tc, out_d.ap(), ht_d.ap(), wq_d.ap(), wsb_d.ap())
    nc.compile()
    _CACHE["nc"] = nc
    return nc


# Device column i holds node pi(i) = 4*(i % 128) + i // 128, so that the
# attention output tile [128p, NC, D] is node-ordered after a host reshape
# (node = 4p + mc) and the output DMA has 1KB-contiguous lines.
_PERM = (np.arange(M) % 128) * NC + (np.arange(M) // 128)


def host_prep(h, W, a):
    wt = W.T.astype(np.float32)  # [128, 64]
    w_src = wt @ a[:D]
    w_dst = wt @ a[D:]
    wq = np.concatenate(
        [wt, w_dst[:, None], 0.2 * w_dst[:, None]], axis=1
    ).astype(ml_dtypes.bfloat16)  # [128, 66]
    wsb = np.ascontiguousarray(
        np.repeat((-0.8 * w_src)[:, None], 128, axis=1)
    ).astype(ml_dtypes.bfloat16)  # [128, 128], every column -0.8*w_src
    return wq, wsb


def kernel(h, W, a):
    global LAST_RESULTS
    h = np.asarray(h, dtype=np.float32)
    W = np.asarray(W, dtype=np.float32)
    a = np.asarray(a, dtype=np.float32)

    wq, wsb = host_prep(h, W, a)

    nc = _build()
    in_maps = []
    for c in range(N_CORES):
        h_c = h[c * G : (c + 1) * G]  # [G, 512, 128]
        ht_c = h_c[:, _PERM, :].transpose(0, 2, 1)  # [G, 128, 512]
        ht_p = np.ascontiguousarray(
            ht_c.reshape(NP, 2, IN_DIM, M).transpose(0, 2, 1, 3)
        ).astype(ml_dtypes.bfloat16)  # [NP, 128, 2, 512]
        in_maps.append({"ht": ht_p, "wq": wq, "wsb": wsb})

    res = run_bass_kernel_spmd(nc, in_maps, list(range(N_CORES)))
    LAST_RESULTS = res
    outs = []
    for r in res.results:
        o = np.asarray(r["out"]).astype(np.float32)  # [NP, 128, 2, NC, D]
        outs.append(o.transpose(0, 2, 1, 3, 4).reshape(G, M, D))
    return np.concatenate(outs, axis=0).astype(np.float32)
